# revision 1
# baseline (speedup 1.0000x reference)
"""XNOR-Net++ 3x3 conv (sign(x) (*) sign(w) * alpha*beta*gamma) on 8 TRN2 NeuronCores.

Sharding: data-parallel over batch (32 -> 4 per core), weights/scales replicated.

Per core (measured 176 us HW exec, exact vs fp32 reference):
- binarize x and w on-device to fp8e4 (+-1 is exact; PSUM accumulates fp32 exactly)
- sign images stored as three x-shifted contiguous fp8 copies (one per kx tap),
  each [128, 2, 58, 56], so the DoubleRow rhs AP is exactly [K=128, 2, N=448]
- 3x3 conv = 9 accumulating DoubleRow matmuls per [128, 448] output tile
  (K=256 via input-channel-block pairing, 2 fp8 weights/PE cell)
- weights transposed on-device via PE transpose; pair dim step 128 B (%16==0)
- epilogue: alpha per-channel scale on ACT, beta*gamma per-pixel map on DVE
"""

from contextlib import ExitStack

import numpy as np

import concourse.bacc as bacc
import concourse.bass as bass
import concourse.mybir as mybir
import concourse.tile as tile
from concourse import masks
from concourse.bass_utils import run_bass_kernel_spmd

N_CORES = 8
B, C, H, KS = 32, 256, 56, 3
P = 128
CB = C // P  # input-channel blocks (2)
OB = C // P  # output-channel blocks (2)
HP = H + 2   # padded image rows (58)
R = 8        # output rows per matmul tile
T = H // R   # row tiles per image (7)
NT = R * H   # moving free dim per matmul (448)
HW = H * H   # pixels per image (3136)

F32 = mybir.dt.float32
BF16 = mybir.dt.bfloat16
FP8 = mybir.dt.float8e4
DR = mybir.MatmulPerfMode.DoubleRow


def build_conv(tc, out_ap, x_ap, w_ap, a_ap, b_ap, g_ap, BL):
    nc = tc.nc
    with ExitStack() as ctx:
        const_pool = ctx.enter_context(tc.tile_pool(name="const", bufs=1))
        wpool = ctx.enter_context(tc.tile_pool(name="w", bufs=1))
        xpool = ctx.enter_context(tc.tile_pool(name="x", bufs=2))
        imgpool = ctx.enter_context(tc.tile_pool(name="img", bufs=2))
        psumpool = ctx.enter_context(tc.tile_pool(name="psum", bufs=4, space="PSUM"))
        tpool = ctx.enter_context(tc.tile_pool(name="tmp", bufs=4))
        opool = ctx.enter_context(tc.tile_pool(name="o", bufs=4))

        ident = const_pool.tile([P, P], BF16, name="ident")
        masks.make_identity(nc, ident)

        # ---- weights: load, binarize, transpose, convert to fp8 ----
        w_f32 = wpool.tile([P, OB, C * KS * KS], F32, name="w_f32")
        nc.sync.dma_start(
            w_f32, w_ap.rearrange("(ob p) i ky kx -> p ob (i ky kx)", p=P)
        )
        w_sgn = wpool.tile([P, OB, C * KS * KS], BF16, name="w_sgn")
        nc.scalar.sign(w_sgn, w_f32)
        w_view = w_sgn.rearrange("p ob (i kk) -> p ob kk i", kk=KS * KS)

        # wT2[i_low, tap, ob, cb, o] in fp8; pair dim cb has byte-step 128 (%16==0)
        wT2 = wpool.tile([P, KS * KS, OB, CB, P], FP8, name="wT2")
        for ob in range(OB):
            for ib in range(CB):
                for kk in range(KS * KS):
                    pt = psumpool.tile([P, P], BF16, name="pt", tag="pt", bufs=2)
                    nc.tensor.transpose(
                        pt, w_view[:, ob, kk, ib * P : (ib + 1) * P], ident
                    )
                    nc.scalar.copy(wT2[:, kk, ob, ib, :], pt)

        # ---- scales ----
        a_t = const_pool.tile([P, OB], F32, name="a_t")
        nc.sync.dma_start(a_t, a_ap.rearrange("(ob p) u v -> p (ob u v)", p=P))
        b_t = const_pool.tile([1, H], F32, name="b_t")
        nc.sync.dma_start(b_t, b_ap[0:1, :, 0])
        g_t = const_pool.tile([1, H], F32, name="g_t")
        nc.sync.dma_start(g_t, g_ap[0:1, 0, :])

        # bg_row[0, i*56+j] = beta[i] * gamma[j] — one DVE op, step-0 broadcast reads
        bg_row = const_pool.tile([1, HW], F32, name="bg_row")
        b_rep = b_t[0:1, :].unsqueeze(2).to_broadcast((1, H, H))
        g_rep = g_t[0:1, :].unsqueeze(1).to_broadcast((1, H, H))
        nc.vector.tensor_mul(bg_row.rearrange("a (i j) -> a i j", i=H), b_rep, g_rep)
        ones_t = const_pool.tile([1, P], F32, name="ones_t")
        nc.gpsimd.memset(ones_t, 1.0)
        # broadcast to all 128 partitions via K=1 matmul
        bg_bcast = const_pool.tile([P, HW], F32, name="bg_bcast")
        for t in range(T):
            sl = slice(t * NT, (t + 1) * NT)
            bgp = psumpool.tile([P, NT], F32, name="bgp", tag="bgp", bufs=2)
            nc.tensor.matmul(bgp, ones_t, bg_row[0:1, sl], start=True, stop=True)
            nc.scalar.copy(bg_bcast[:, sl], bgp)

        # ---- main loop over local batches ----
        x_v = x_ap.rearrange("b (cb p) h w -> b p cb (h w)", p=P)
        out_v = out_ap.rearrange("b (ob p) h w -> b ob p (h w)", p=P)
        for b in range(BL):
            x_t = xpool.tile([P, CB, HW], F32, name="x_t")
            nc.sync.dma_start(x_t, x_v[b])
            # im[kx][p, cb, y, j] = padded_sign[p, cb, y, j + kx]
            im1 = imgpool.tile([P, CB, HP, H], FP8, name="im1", tag="im1")
            im0 = imgpool.tile([P, CB, HP, H], FP8, name="im0", tag="im0")
            im2 = imgpool.tile([P, CB, HP, H], FP8, name="im2", tag="im2")
            nc.gpsimd.memset(im1, 0.0)
            nc.gpsimd.memset(im0, 0.0)
            nc.gpsimd.memset(im2, 0.0)
            # kx=1: no column shift — interior rows get the full sign image
            nc.scalar.sign(
                im1[:, :, 1 : H + 1, :],
                x_t.rearrange("p cb (h w) -> p cb h w", h=H),
            )
            # kx=0: right-shift (left pad col enters at j=0)
            nc.vector.tensor_copy(
                im0[:, :, 1 : H + 1, 1:H], im1[:, :, 1 : H + 1, 0 : H - 1]
            )
            # kx=2: left-shift (right pad col at j=H-1)
            nc.vector.tensor_copy(
                im2[:, :, 1 : H + 1, 0 : H - 1], im1[:, :, 1 : H + 1, 1:H]
            )
            ims = [im0, im1, im2]
            for ob in range(OB):
                for t in range(T):
                    ps = psumpool.tile([P, NT], F32, name="cps", tag="cps", bufs=4)
                    for kk in range(KS * KS):
                        ky, kx = divmod(kk, KS)
                        rhs = ims[kx][:, :, t * R + ky : t * R + ky + R, :]
                        nc.tensor.matmul(
                            ps,
                            wT2[:, kk, ob, :, :],
                            rhs,
                            start=(kk == 0),
                            stop=(kk == KS * KS - 1),
                            perf_mode=DR,
                        )
                    sl = slice(t * NT, (t + 1) * NT)
                    tmp = tpool.tile([P, NT], F32, name="tmp")
                    nc.scalar.mul(tmp, ps, a_t[:, ob : ob + 1])
                    ot = opool.tile([P, NT], F32, name="ot")
                    nc.vector.tensor_mul(ot, tmp, bg_bcast[:, sl])
                    nc.sync.dma_start(out_v[b, ob][:, sl], ot)


def build_nc(BL):
    nc = bacc.Bacc("TRN2", target_bir_lowering=False, debug=False)
    x = nc.dram_tensor("x", [BL, C, H, H], F32, kind="ExternalInput")
    w = nc.dram_tensor("weight", [C, C, KS, KS], F32, kind="ExternalInput")
    a = nc.dram_tensor("alpha", [C, 1, 1], F32, kind="ExternalInput")
    be = nc.dram_tensor("beta", [1, H, 1], F32, kind="ExternalInput")
    g = nc.dram_tensor("gamma", [1, 1, H], F32, kind="ExternalInput")
    o = nc.dram_tensor("out", [BL, C, H, H], F32, kind="ExternalOutput")
    with tile.TileContext(nc) as tc:
        build_conv(tc, o.ap(), x.ap(), w.ap(), a.ap(), be.ap(), g.ap(), BL)
    nc.compile()
    return nc


_nc_cache = {}


def _get_nc(BL):
    if BL not in _nc_cache:
        _nc_cache[BL] = build_nc(BL)
    return _nc_cache[BL]


def kernel(x, weight, alpha, beta, gamma):
    x = np.ascontiguousarray(np.asarray(x, dtype=np.float32))
    weight = np.ascontiguousarray(np.asarray(weight, dtype=np.float32))
    alpha = np.ascontiguousarray(np.asarray(alpha, dtype=np.float32))
    beta = np.ascontiguousarray(np.asarray(beta, dtype=np.float32))
    gamma = np.ascontiguousarray(np.asarray(gamma, dtype=np.float32))

    BL = B // N_CORES
    nc = _get_nc(BL)
    xs = x.reshape(N_CORES, BL, C, H, H)
    in_maps = [
        {"x": xs[c], "weight": weight, "alpha": alpha, "beta": beta, "gamma": gamma}
        for c in range(N_CORES)
    ]
    res = run_bass_kernel_spmd(nc, in_maps, list(range(N_CORES)))
    return np.concatenate([r["out"] for r in res.results], axis=0)



# revision 2
# speedup vs baseline: 1.1776x; 1.1776x over previous
"""XNOR-Net++ 3x3 conv (sign(x) (*) sign(w) * alpha*beta*gamma) on 8 TRN2 NeuronCores.

Sharding: data-parallel over batch (32 -> 4 per core), weights/scales replicated.

Per core:
- weights arrive pre-transposed from host ([i, ky*kx, ob, cb, o] layout, f32),
  binarized on-device to fp8 in one ACT op (no PE transposes)
- sign image: ONE padded fp8 buffer [128, 2, 58, 64] per image (double-buffered,
  borders zeroed once in the preamble); the 9 conv taps read strided windows
  [*, *, t*8+ky : +8, kx : kx+56] directly -- no shifted copies, no per-image memsets
- per image: 2 DMAs (top/bottom rows) + 2 ACT sign ops; sub-tile dep tracking lets
  the first row-tiles' matmuls start as soon as the top half is signed
- 3x3 conv = 9 accumulating DoubleRow fp8 matmuls per [128, 448] output tile
  (K=256 via input-channel-block pairing, 2 fp8 weights/PE cell)
- epilogue: single DVE mul with a precomputed alpha*beta*gamma map
  (built via K=1 broadcast matmuls in the preamble)
"""

from contextlib import ExitStack

import numpy as np

import concourse.bacc as bacc
import concourse.bass as bass
import concourse.mybir as mybir
import concourse.tile as tile
from concourse.bass_utils import run_bass_kernel_spmd

N_CORES = 8
B, C, H, KS = 32, 256, 56, 3
P = 128
CB = C // P  # input-channel blocks (2)
OB = C // P  # output-channel blocks (2)
HP = H + 2   # padded image rows (58)
WP = 64      # padded image row pitch (cols 0..57 live, 58..63 never read)
R = 8        # output rows per matmul tile
T = H // R   # row tiles per image (7)
NT = R * H   # moving free dim per matmul (448)
HW = H * H   # pixels per image (3136)
RT = 33      # top-half data rows (0..32) -> padded rows 1..33 (t=0..3 need <=33)
RB = H - RT  # bottom-half data rows (33..55) -> padded rows 34..56

F32 = mybir.dt.float32
FP8 = mybir.dt.float8e4
DR = mybir.MatmulPerfMode.DoubleRow


def build_conv(tc, out_ap, x_ap, wT_ap, a_ap, b_ap, g_ap, BL):
    nc = tc.nc
    with ExitStack() as ctx:
        const_pool = ctx.enter_context(tc.tile_pool(name="const", bufs=1))
        wpool = ctx.enter_context(tc.tile_pool(name="w", bufs=1))
        xpool = ctx.enter_context(tc.tile_pool(name="x", bufs=2))
        impool = ctx.enter_context(tc.tile_pool(name="img", bufs=1))
        psumpool = ctx.enter_context(tc.tile_pool(name="psum", bufs=4, space="PSUM"))
        opool = ctx.enter_context(tc.tile_pool(name="o", bufs=4))

        # ---- weights: load pre-transposed f32, binarize to fp8 ----
        w_f32 = wpool.tile([P, KS * KS * OB * CB * P], F32, name="w_f32")
        nc.sync.dma_start(w_f32, wT_ap)
        wT2 = wpool.tile([P, KS * KS * OB * CB * P], FP8, name="wT2")
        nc.scalar.sign(wT2, w_f32)
        # wv[i_low, tap, ob, cb, o]; pair dim cb has byte-step 128 (%16==0)
        wv = wT2.rearrange("p (kk ob cb o) -> p kk ob cb o", kk=KS * KS, ob=OB, cb=CB)

        # ---- scales: abg[o_low, ob, pix] = alpha[o] * beta[i] * gamma[j] ----
        a_row = const_pool.tile([1, C], F32, name="a_row")
        nc.sync.dma_start(a_row, a_ap.rearrange("o u v -> u (o v)"))
        b_t = const_pool.tile([1, H], F32, name="b_t")
        nc.sync.dma_start(b_t, b_ap[0:1, :, 0])
        g_t = const_pool.tile([1, H], F32, name="g_t")
        nc.sync.dma_start(g_t, g_ap[0:1, 0, :])

        # bg_row[0, i*56+j] = beta[i] * gamma[j] -- one DVE op, step-0 broadcast reads
        bg_row = const_pool.tile([1, HW], F32, name="bg_row")
        b_rep = b_t[0:1, :].unsqueeze(2).to_broadcast((1, H, H))
        g_rep = g_t[0:1, :].unsqueeze(1).to_broadcast((1, H, H))
        nc.vector.tensor_mul(bg_row.rearrange("a (i j) -> a i j", i=H), b_rep, g_rep)

        # broadcast to 128 partitions with the alpha scale folded in:
        # K=1 matmul out[o, n] = alpha[ob*128+o] * bg_row[n]
        abg = const_pool.tile([P, OB, HW], F32, name="abg")
        for ob in range(OB):
            for t in range(T):
                sl = slice(t * NT, (t + 1) * NT)
                bgp = psumpool.tile([P, NT], F32, name="bgp", tag="bgp", bufs=2)
                nc.tensor.matmul(
                    bgp,
                    a_row[0:1, ob * P : (ob + 1) * P],
                    bg_row[0:1, sl],
                    start=True,
                    stop=True,
                )
                nc.vector.tensor_copy(abg[:, ob, sl], bgp)

        # ---- persistent padded sign images; borders zeroed once ----
        ims = [
            impool.tile([P, CB, HP, WP], FP8, name=f"im{i}", tag=f"im{i}")
            for i in range(2)
        ]
        for im in ims:
            nc.gpsimd.memset(im[:, :, 0, 0:58], 0.0)
            nc.gpsimd.memset(im[:, :, HP - 1, 0:58], 0.0)
            nc.gpsimd.memset(im[:, :, 1 : HP - 1, 0], 0.0)
            nc.gpsimd.memset(im[:, :, 1 : HP - 1, 57], 0.0)

        # ---- main loop over local batches ----
        x_v = x_ap.rearrange("b (cb p) h w -> b p cb (h w)", p=P)
        out_v = out_ap.rearrange("b (ob p) h w -> b ob p (h w)", p=P)
        for b in range(BL):
            im = ims[b % 2]
            xT = xpool.tile([P, CB, RT * H], F32, name="xT", tag="xT")
            nc.sync.dma_start(xT, x_v[b][:, :, 0 : RT * H])
            xB = xpool.tile([P, CB, RB * H], F32, name="xB", tag="xB")
            nc.sync.dma_start(xB, x_v[b][:, :, RT * H : HW])
            nc.scalar.sign(
                im[:, :, 1 : 1 + RT, 1 : 1 + H],
                xT.rearrange("p c (h w) -> p c h w", h=RT),
            )
            nc.scalar.sign(
                im[:, :, 1 + RT : 1 + H, 1 : 1 + H],
                xB.rearrange("p c (h w) -> p c h w", h=RB),
            )
            for ob in range(OB):
                for t in range(T):
                    ps = psumpool.tile([P, NT], F32, name="cps", tag="cps", bufs=4)
                    for kk in range(KS * KS):
                        ky, kx = divmod(kk, KS)
                        rhs = im[:, :, t * R + ky : t * R + ky + R, kx : kx + H]
                        nc.tensor.matmul(
                            ps,
                            wv[:, kk, ob],
                            rhs,
                            start=(kk == 0),
                            stop=(kk == KS * KS - 1),
                            perf_mode=DR,
                        )
                    sl = slice(t * NT, (t + 1) * NT)
                    ot = opool.tile([P, NT], F32, name="ot")
                    nc.vector.tensor_mul(ot, ps, abg[:, ob, sl])
                    nc.sync.dma_start(out_v[b, ob][:, sl], ot)


def build_nc(BL):
    nc = bacc.Bacc("TRN2", target_bir_lowering=False, debug=False)
    x = nc.dram_tensor("x", [BL, C, H, H], F32, kind="ExternalInput")
    wT = nc.dram_tensor(
        "weightT", [P, KS * KS * OB * CB * P], F32, kind="ExternalInput"
    )
    a = nc.dram_tensor("alpha", [C, 1, 1], F32, kind="ExternalInput")
    be = nc.dram_tensor("beta", [1, H, 1], F32, kind="ExternalInput")
    g = nc.dram_tensor("gamma", [1, 1, H], F32, kind="ExternalInput")
    o = nc.dram_tensor("out", [BL, C, H, H], F32, kind="ExternalOutput")
    with tile.TileContext(nc) as tc:
        build_conv(tc, o.ap(), x.ap(), wT.ap(), a.ap(), be.ap(), g.ap(), BL)
    nc.compile()
    return nc


_nc_cache = {}


def _get_nc(BL):
    if BL not in _nc_cache:
        _nc_cache[BL] = build_nc(BL)
    return _nc_cache[BL]


def _prep(x, weight, alpha, beta, gamma):
    """Build the bass kernel and the per-core input maps."""
    x = np.ascontiguousarray(np.asarray(x, dtype=np.float32))
    weight = np.ascontiguousarray(np.asarray(weight, dtype=np.float32))
    alpha = np.ascontiguousarray(np.asarray(alpha, dtype=np.float32))
    beta = np.ascontiguousarray(np.asarray(beta, dtype=np.float32))
    gamma = np.ascontiguousarray(np.asarray(gamma, dtype=np.float32))

    # [o, i, ky, kx] -> [i_low, (ky kx), ob, cb, o_low]
    w6 = weight.reshape(OB, P, CB, P, KS, KS)
    wT = np.ascontiguousarray(w6.transpose(3, 4, 5, 0, 2, 1)).reshape(
        P, KS * KS * OB * CB * P
    )

    BL = B // N_CORES
    nc = _get_nc(BL)
    xs = x.reshape(N_CORES, BL, C, H, H)
    in_maps = [
        {"x": xs[c], "weightT": wT, "alpha": alpha, "beta": beta, "gamma": gamma}
        for c in range(N_CORES)
    ]
    return nc, in_maps


def kernel(x, weight, alpha, beta, gamma):
    nc, in_maps = _prep(x, weight, alpha, beta, gamma)
    res = run_bass_kernel_spmd(nc, in_maps, list(range(N_CORES)))
    return np.concatenate([r["out"] for r in res.results], axis=0)


# revision 4
# speedup vs baseline: 1.3750x; 1.1676x over previous
"""XNOR-Net++ 3x3 conv (sign(x) (*) sign(w) * alpha*beta*gamma) on 8 TRN2 NeuronCores.

Sharding: data-parallel over batch (32 -> 4 per core), weights/scales replicated.

Per core:
- x and the pre-transposed weight are staged to HBM as bf16 (sign-preserving cast,
  halves DMA); output is written bf16 and upcast on host (conv values are integers
  <= 2304, bf16 rel err < 0.4% << 2e-2 gate)
- weights arrive pre-transposed from host ([i, ky*kx, ob, cb, o] layout),
  binarized on-device to fp8 in one ACT op (no PE transposes)
- sign image: ONE padded fp8 buffer [128, 2, 58, 64] per image (double-buffered,
  borders zeroed once in the preamble); the 9 conv taps read strided windows
  [*, *, t*8+ky : +8, kx : kx+56] directly -- no shifted copies, no per-image
  memsets; sign runs in 7 row-chunks so early row-tiles' matmuls start ASAP
- PE warm-up filler matmuls bridge the DMA-bound startup so the HAM clock gate
  stays at 2.4 GHz when the conv stream begins (cold-start costs ~25us otherwise)
- 3x3 conv = 9 accumulating DoubleRow fp8 matmuls per [128, 448] output tile
  (K=256 via input-channel-block pairing, 2 fp8 weights/PE cell)
- epilogue: single DVE mul with a precomputed alpha*beta*gamma map
  (partition_broadcast + per-partition alpha scale; no fp32 matmuls)
"""

from contextlib import ExitStack

import ml_dtypes
import numpy as np

import concourse.bacc as bacc
import concourse.bass as bass
import concourse.mybir as mybir
import concourse.tile as tile
from concourse.bass_utils import run_bass_kernel_spmd

N_CORES = 8
B, C, H, KS = 32, 256, 56, 3
P = 128
CB = C // P  # input-channel blocks (2)
OB = C // P  # output-channel blocks (2)
HP = H + 2   # padded image rows (58)
WP = 64      # padded image row pitch (cols 0..57 live, 58..63 never read)
R = 8        # output rows per matmul tile
T = H // R   # row tiles per image (7)
NT = R * H   # moving free dim per matmul (448)
HW = H * H   # pixels per image (3136)
RT = 33      # top-half data rows (0..32); bottom half is rows 33..55
RB = H - RT
# sign row-chunks (data-row ranges); first four read the top x tile
CHUNKS = [(0, 9), (9, 17), (17, 25), (25, 33), (33, 41), (41, 49), (49, 56)]
N_WARM = 44  # PE warm-up fillers bridging the DMA-bound startup

F32 = mybir.dt.float32
BF16 = mybir.dt.bfloat16
FP8 = mybir.dt.float8e4
DR = mybir.MatmulPerfMode.DoubleRow


def build_conv(tc, out_ap, x_ap, wT_ap, a_ap, b_ap, g_ap, BL):
    nc = tc.nc
    with ExitStack() as ctx:
        const_pool = ctx.enter_context(tc.tile_pool(name="const", bufs=1))
        wpool = ctx.enter_context(tc.tile_pool(name="w", bufs=1))
        xpool = ctx.enter_context(tc.tile_pool(name="x", bufs=2))
        impool = ctx.enter_context(tc.tile_pool(name="img", bufs=1))
        psumpool = ctx.enter_context(tc.tile_pool(name="psum", bufs=4, space="PSUM"))
        opool = ctx.enter_context(tc.tile_pool(name="o", bufs=4))

        # ---- tiny scale DMAs first so they aren't stuck behind the big ones ----
        a_t = const_pool.tile([P, OB], F32, name="a_t")
        nc.sync.dma_start(a_t, a_ap.rearrange("(ob p) u v -> p (ob u v)", p=P))
        b_t = const_pool.tile([1, H], F32, name="b_t")
        nc.sync.dma_start(b_t, b_ap[0:1, :, 0])
        g_t = const_pool.tile([1, H], F32, name="g_t")
        nc.sync.dma_start(g_t, g_ap[0:1, 0, :])

        # ---- weights: load pre-transposed bf16, binarize to fp8 ----
        w_bf = wpool.tile([P, KS * KS * OB * CB * P], BF16, name="w_bf")
        nc.sync.dma_start(w_bf, wT_ap)
        wT2 = wpool.tile([P, KS * KS * OB * CB * P], FP8, name="wT2")
        nc.scalar.sign(wT2, w_bf)
        # wv[i_low, tap, ob, cb, o]; pair dim cb has byte-step 128 (%16==0)
        wv = wT2.rearrange("p (kk ob cb o) -> p kk ob cb o", kk=KS * KS, ob=OB, cb=CB)

        # bg_row[0, i*56+j] = beta[i] * gamma[j] -- one DVE op, step-0 broadcast reads
        bg_row = const_pool.tile([1, HW], BF16, name="bg_row")
        b_rep = b_t[0:1, :].unsqueeze(2).to_broadcast((1, H, H))
        g_rep = g_t[0:1, :].unsqueeze(1).to_broadcast((1, H, H))
        nc.vector.tensor_mul(bg_row.rearrange("a (i j) -> a i j", i=H), b_rep, g_rep)

        # ---- PE warm-up fillers: keep HAM at 2.4 GHz until the conv stream ----
        ones_t = const_pool.tile([1, NT], BF16, name="ones_t")
        nc.gpsimd.memset(ones_t, 1.0)
        warm_ps = psumpool.tile([P, NT], F32, name="warm", tag="warm", bufs=1)
        for _ in range(N_WARM):
            nc.tensor.matmul(
                warm_ps, ones_t[0:1, 0:P], ones_t, start=True, stop=True
            )

        # ---- persistent padded sign images; borders zeroed once ----
        ims = [
            impool.tile([P, CB, HP, WP], FP8, name=f"im{i}", tag=f"im{i}")
            for i in range(2)
        ]
        for im in ims:
            nc.gpsimd.memset(im[:, :, 0, 0:58], 0.0)
            nc.gpsimd.memset(im[:, :, HP - 1, 0:58], 0.0)
            nc.gpsimd.memset(im[:, :, 1 : HP - 1, 0], 0.0)
            nc.gpsimd.memset(im[:, :, 1 : HP - 1, 57], 0.0)

        abg = const_pool.tile([P, OB, HW], BF16, name="abg")
        bg_bc = const_pool.tile([P, HW], BF16, name="bg_bc")

        # ---- main loop over local batches ----
        x_v = x_ap.rearrange("b (cb p) h w -> b p cb (h w)", p=P)
        out_v = out_ap.rearrange("b (ob p) h w -> b ob p (h w)", p=P)
        for b in range(BL):
            im = ims[b % 2]
            xT = xpool.tile([P, CB, RT * H], BF16, name="xT", tag="xT")
            nc.sync.dma_start(xT, x_v[b][:, :, 0 : RT * H])
            xB = xpool.tile([P, CB, RB * H], BF16, name="xB", tag="xB")
            nc.sync.dma_start(xB, x_v[b][:, :, RT * H : HW])
            xTv = xT.rearrange("p c (h w) -> p c h w", h=RT)
            xBv = xB.rearrange("p c (h w) -> p c h w", h=RB)
            for r0, r1 in CHUNKS:
                src = (
                    xTv[:, :, r0:r1, :]
                    if r1 <= RT
                    else xBv[:, :, r0 - RT : r1 - RT, :]
                )
                nc.scalar.sign(im[:, :, 1 + r0 : 1 + r1, 1 : 1 + H], src)

            if b == 0:
                # alpha*beta*gamma map: broadcast bg to 128 partitions, fold alpha.
                # Emitted after image-0's DMAs so the Sync engine doesn't stall
                # the x dispatches waiting for bg_row.
                nc.gpsimd.partition_broadcast(bg_bc, bg_row)
                for ob in range(OB):
                    nc.vector.tensor_scalar_mul(
                        abg[:, ob, :], bg_bc, a_t[:, ob : ob + 1]
                    )

            for ob in range(OB):
                for t in range(T):
                    ps = psumpool.tile([P, NT], F32, name="cps", tag="cps", bufs=4)
                    for kk in range(KS * KS):
                        ky, kx = divmod(kk, KS)
                        rhs = im[:, :, t * R + ky : t * R + ky + R, kx : kx + H]
                        nc.tensor.matmul(
                            ps,
                            wv[:, kk, ob],
                            rhs,
                            start=(kk == 0),
                            stop=(kk == KS * KS - 1),
                            perf_mode=DR,
                        )
                    sl = slice(t * NT, (t + 1) * NT)
                    ot = opool.tile([P, NT], BF16, name="ot")
                    nc.vector.tensor_mul(ot, ps, abg[:, ob, sl])
                    nc.sync.dma_start(out_v[b, ob][:, sl], ot)


def build_nc(BL):
    nc = bacc.Bacc("TRN2", target_bir_lowering=False, debug=False)
    x = nc.dram_tensor("x", [BL, C, H, H], BF16, kind="ExternalInput")
    wT = nc.dram_tensor(
        "weightT", [P, KS * KS * OB * CB * P], BF16, kind="ExternalInput"
    )
    a = nc.dram_tensor("alpha", [C, 1, 1], F32, kind="ExternalInput")
    be = nc.dram_tensor("beta", [1, H, 1], F32, kind="ExternalInput")
    g = nc.dram_tensor("gamma", [1, 1, H], F32, kind="ExternalInput")
    o = nc.dram_tensor("out", [BL, C, H, H], BF16, kind="ExternalOutput")
    with tile.TileContext(nc) as tc:
        build_conv(tc, o.ap(), x.ap(), wT.ap(), a.ap(), be.ap(), g.ap(), BL)
    nc.compile()
    return nc


_nc_cache = {}


def _get_nc(BL):
    if BL not in _nc_cache:
        _nc_cache[BL] = build_nc(BL)
    return _nc_cache[BL]


def _prep(x, weight, alpha, beta, gamma):
    """Build the bass kernel and the per-core input maps."""
    x = np.asarray(x, dtype=np.float32)
    weight = np.asarray(weight, dtype=np.float32)
    alpha = np.ascontiguousarray(np.asarray(alpha, dtype=np.float32))
    beta = np.ascontiguousarray(np.asarray(beta, dtype=np.float32))
    gamma = np.ascontiguousarray(np.asarray(gamma, dtype=np.float32))

    # bf16 staging: sign(bf16(v)) == sign(v) for all practically occurring values
    x_bf = np.ascontiguousarray(x.astype(ml_dtypes.bfloat16))
    # [o, i, ky, kx] -> [i_low, (ky kx), ob, cb, o_low]
    w6 = weight.reshape(OB, P, CB, P, KS, KS)
    wT = np.ascontiguousarray(
        w6.transpose(3, 4, 5, 0, 2, 1).astype(ml_dtypes.bfloat16)
    ).reshape(P, KS * KS * OB * CB * P)

    BL = B // N_CORES
    nc = _get_nc(BL)
    xs = x_bf.reshape(N_CORES, BL, C, H, H)
    in_maps = [
        {"x": xs[c], "weightT": wT, "alpha": alpha, "beta": beta, "gamma": gamma}
        for c in range(N_CORES)
    ]
    return nc, in_maps


def kernel(x, weight, alpha, beta, gamma):
    nc, in_maps = _prep(x, weight, alpha, beta, gamma)
    res = run_bass_kernel_spmd(nc, in_maps, list(range(N_CORES)))
    out = np.concatenate([r["out"] for r in res.results], axis=0)
    return out.astype(np.float32)


# revision 8
# speedup vs baseline: 1.4453x; 1.0511x over previous
"""XNOR-Net++ 3x3 conv (sign(x) (*) sign(w) * alpha*beta*gamma) on 8 TRN2 NeuronCores.

Sharding: data-parallel over batch (32 -> 4 per core), weights/scales replicated.

Per core:
- x and the pre-transposed weight are staged to HBM as bf16 (sign-preserving cast,
  halves DMA); output is written bf16 and upcast on host (conv values are integers
  <= 2304, bf16 rel err < 0.4% << 2e-2 gate)
- weights arrive pre-transposed from host ([i, ky*kx, ob, cb, o] layout),
  binarized on-device to fp8 in one ACT op (no PE transposes)
- sign image: ONE padded fp8 buffer [128, 2, 58, 64] per image (double-buffered,
  borders zeroed once in the preamble); the 9 conv taps read strided windows
  [*, *, t*8+ky : +8, kx : kx+56] directly -- no shifted copies, no per-image
  memsets; sign runs in 7 row-chunks so early row-tiles' matmuls start ASAP
- PE warm-up filler matmuls bridge the DMA-bound startup so the HAM clock gate
  stays at 2.4 GHz when the conv stream begins (cold-start costs ~25us otherwise)
- 3x3 conv = 9 accumulating DoubleRow fp8 matmuls per [128, 448] output tile
  (K=256 via input-channel-block pairing, 2 fp8 weights/PE cell)
- epilogue: single DVE mul with a precomputed alpha*beta*gamma map
  (partition_broadcast + per-partition alpha scale; no fp32 matmuls)
"""

from contextlib import ExitStack

import ml_dtypes
import numpy as np

import concourse.bacc as bacc
import concourse.bass as bass
import concourse.mybir as mybir
import concourse.tile as tile
from concourse.bass_utils import run_bass_kernel_spmd

N_CORES = 8
B, C, H, KS = 32, 256, 56, 3
P = 128
CB = C // P  # input-channel blocks (2)
OB = C // P  # output-channel blocks (2)
HP = H + 2   # padded image rows (58)
WP = 64      # padded image row pitch (cols 0..57 live, 58..63 never read)
R = 8        # output rows per matmul tile
T = H // R   # row tiles per image (7)
NT = R * H   # moving free dim per matmul (448)
HW = H * H   # pixels per image (3136)
RA = 9       # first x chunk (data rows 0..8) -- lands early, unblocks tile t=0
RT = 24      # second x chunk (data rows 9..32)
RB = H - RA - RT  # third x chunk (data rows 33..55)
# sign row-chunks (data-row ranges); chunk 0 reads xA, 1-3 read xT, 4-6 read xB
CHUNKS = [(0, 9), (9, 17), (17, 25), (25, 33), (33, 41), (41, 49), (49, 56)]
N_WARM = 26  # PE warm-up fillers bridging the DMA-bound startup
WSPLIT = 2048  # weight sign chunk boundary (taps 0-3 | 4-8), overlaps w DMA

F32 = mybir.dt.float32
BF16 = mybir.dt.bfloat16
FP8 = mybir.dt.float8e4
DR = mybir.MatmulPerfMode.DoubleRow


def build_conv(tc, out_ap, x_ap, wT_ap, a_ap, b_ap, g_ap, BL):
    nc = tc.nc
    with ExitStack() as ctx:
        const_pool = ctx.enter_context(tc.tile_pool(name="const", bufs=1))
        wpool = ctx.enter_context(tc.tile_pool(name="w", bufs=1))
        xpool = ctx.enter_context(tc.tile_pool(name="x", bufs=2))
        impool = ctx.enter_context(tc.tile_pool(name="img", bufs=1))
        psumpool = ctx.enter_context(tc.tile_pool(name="psum", bufs=4, space="PSUM"))
        opool = ctx.enter_context(tc.tile_pool(name="o", bufs=4))

        # ---- weights first (the startup-critical DMA), split so the ACT sign
        # of taps 0-3 overlaps the DMA of taps 4-8 ----
        w_bf = wpool.tile([P, KS * KS * OB * CB * P], BF16, name="w_bf")
        nc.sync.dma_start(w_bf[:, 0:WSPLIT], wT_ap[:, 0:WSPLIT])
        nc.sync.dma_start(
            w_bf[:, WSPLIT : KS * KS * OB * CB * P],
            wT_ap[:, WSPLIT : KS * KS * OB * CB * P],
        )
        wT2 = wpool.tile([P, KS * KS * OB * CB * P], FP8, name="wT2")
        nc.scalar.sign(wT2[:, 0:WSPLIT], w_bf[:, 0:WSPLIT])
        nc.scalar.sign(
            wT2[:, WSPLIT : KS * KS * OB * CB * P],
            w_bf[:, WSPLIT : KS * KS * OB * CB * P],
        )
        # wv[i_low, tap, ob, cb, o]; pair dim cb has byte-step 128 (%16==0)
        wv = wT2.rearrange("p (kk ob cb o) -> p kk ob cb o", kk=KS * KS, ob=OB, cb=CB)

        # ---- PE warm-up fillers: keep HAM at 2.4 GHz until the conv stream ----
        ones_t = const_pool.tile([1, NT], BF16, name="ones_t")
        nc.gpsimd.memset(ones_t, 1.0)
        warm_ps = psumpool.tile([P, NT], F32, name="warm", tag="warm", bufs=1)
        for _ in range(N_WARM):
            nc.tensor.matmul(
                warm_ps, ones_t[0:1, 0:P], ones_t, start=True, stop=True
            )

        # ---- persistent padded sign images; borders zeroed once ----
        ims = [
            impool.tile([P, CB, HP, WP], FP8, name=f"im{i}", tag=f"im{i}")
            for i in range(2)
        ]
        for im in ims:
            nc.gpsimd.memset(im[:, :, 0, 0:58], 0.0)
            nc.gpsimd.memset(im[:, :, HP - 1, 0:58], 0.0)
            nc.gpsimd.memset(im[:, :, 1 : HP - 1, 0], 0.0)
            nc.gpsimd.memset(im[:, :, 1 : HP - 1, 57], 0.0)

        abg = const_pool.tile([P, OB, HW], BF16, name="abg")
        bg_bc = const_pool.tile([P, HW], BF16, name="bg_bc")

        # ---- main loop over local batches ----
        x_v = x_ap.rearrange("b (cb p) h w -> b p cb (h w)", p=P)
        out_v = out_ap.rearrange("b (ob p) h w -> b ob p (h w)", p=P)
        for b in range(BL):
            im = ims[b % 2]
            xA = xpool.tile([P, CB, RA * H], BF16, name="xA", tag="xA")
            nc.sync.dma_start(xA, x_v[b][:, :, 0 : RA * H])
            xT = xpool.tile([P, CB, RT * H], BF16, name="xT", tag="xT")
            nc.sync.dma_start(xT, x_v[b][:, :, RA * H : (RA + RT) * H])
            xB = xpool.tile([P, CB, RB * H], BF16, name="xB", tag="xB")
            nc.sync.dma_start(xB, x_v[b][:, :, (RA + RT) * H : HW])
            views = [
                (0, xA.rearrange("p c (h w) -> p c h w", h=RA)),
                (RA, xT.rearrange("p c (h w) -> p c h w", h=RT)),
                (RA + RT, xB.rearrange("p c (h w) -> p c h w", h=RB)),
            ]
            for r0, r1 in CHUNKS:
                v0, view = next(v for v in reversed(views) if v[0] <= r0)
                nc.scalar.sign(
                    im[:, :, 1 + r0 : 1 + r1, 1 : 1 + H],
                    view[:, :, r0 - v0 : r1 - v0, :],
                )

            if b == 0:
                # tiny scale DMAs + alpha*beta*gamma map. Emitted after the
                # startup-critical dispatches (w, image-0 x) so the Sync engine
                # doesn't delay those; the map is only needed by epilogues, and
                # cps bufs=7 means a late map can't stall the PE.
                a_t = const_pool.tile([P, OB], F32, name="a_t")
                nc.sync.dma_start(
                    a_t, a_ap.rearrange("(ob p) u v -> p (ob u v)", p=P)
                )
                b_t = const_pool.tile([1, H], F32, name="b_t")
                nc.sync.dma_start(b_t, b_ap[0:1, :, 0])
                g_t = const_pool.tile([1, H], F32, name="g_t")
                nc.sync.dma_start(g_t, g_ap[0:1, 0, :])
                bg_row = const_pool.tile([1, HW], BF16, name="bg_row")
                b_rep = b_t[0:1, :].unsqueeze(2).to_broadcast((1, H, H))
                g_rep = g_t[0:1, :].unsqueeze(1).to_broadcast((1, H, H))
                nc.vector.tensor_mul(
                    bg_row.rearrange("a (i j) -> a i j", i=H), b_rep, g_rep
                )
                nc.gpsimd.partition_broadcast(bg_bc, bg_row)
                for ob in range(OB):
                    nc.vector.tensor_scalar_mul(
                        abg[:, ob, :], bg_bc, a_t[:, ob : ob + 1]
                    )

            for ob in range(OB):
                for t in range(T):
                    ps = psumpool.tile([P, NT], F32, name="cps", tag="cps", bufs=7)
                    for kk in range(KS * KS):
                        ky, kx = divmod(kk, KS)
                        rhs = im[:, :, t * R + ky : t * R + ky + R, kx : kx + H]
                        nc.tensor.matmul(
                            ps,
                            wv[:, kk, ob],
                            rhs,
                            start=(kk == 0),
                            stop=(kk == KS * KS - 1),
                            perf_mode=DR,
                        )
                    sl = slice(t * NT, (t + 1) * NT)
                    ot = opool.tile([P, NT], BF16, name="ot")
                    nc.vector.tensor_mul(ot, ps, abg[:, ob, sl])
                    nc.sync.dma_start(out_v[b, ob][:, sl], ot)


def build_nc(BL):
    nc = bacc.Bacc("TRN2", target_bir_lowering=False, debug=False)
    x = nc.dram_tensor("x", [BL, C, H, H], BF16, kind="ExternalInput")
    wT = nc.dram_tensor(
        "weightT", [P, KS * KS * OB * CB * P], BF16, kind="ExternalInput"
    )
    a = nc.dram_tensor("alpha", [C, 1, 1], F32, kind="ExternalInput")
    be = nc.dram_tensor("beta", [1, H, 1], F32, kind="ExternalInput")
    g = nc.dram_tensor("gamma", [1, 1, H], F32, kind="ExternalInput")
    o = nc.dram_tensor("out", [BL, C, H, H], BF16, kind="ExternalOutput")
    with tile.TileContext(nc) as tc:
        build_conv(tc, o.ap(), x.ap(), wT.ap(), a.ap(), be.ap(), g.ap(), BL)
    nc.compile()
    return nc


_nc_cache = {}


def _get_nc(BL):
    if BL not in _nc_cache:
        _nc_cache[BL] = build_nc(BL)
    return _nc_cache[BL]


def _prep(x, weight, alpha, beta, gamma):
    """Build the bass kernel and the per-core input maps."""
    x = np.asarray(x, dtype=np.float32)
    weight = np.asarray(weight, dtype=np.float32)
    alpha = np.ascontiguousarray(np.asarray(alpha, dtype=np.float32))
    beta = np.ascontiguousarray(np.asarray(beta, dtype=np.float32))
    gamma = np.ascontiguousarray(np.asarray(gamma, dtype=np.float32))

    # bf16 staging: sign(bf16(v)) == sign(v) for all practically occurring values
    x_bf = np.ascontiguousarray(x.astype(ml_dtypes.bfloat16))
    # [o, i, ky, kx] -> [i_low, (ky kx), ob, cb, o_low]
    w6 = weight.reshape(OB, P, CB, P, KS, KS)
    wT = np.ascontiguousarray(
        w6.transpose(3, 4, 5, 0, 2, 1).astype(ml_dtypes.bfloat16)
    ).reshape(P, KS * KS * OB * CB * P)

    BL = B // N_CORES
    nc = _get_nc(BL)
    xs = x_bf.reshape(N_CORES, BL, C, H, H)
    in_maps = [
        {"x": xs[c], "weightT": wT, "alpha": alpha, "beta": beta, "gamma": gamma}
        for c in range(N_CORES)
    ]
    return nc, in_maps


def kernel(x, weight, alpha, beta, gamma):
    nc, in_maps = _prep(x, weight, alpha, beta, gamma)
    res = run_bass_kernel_spmd(nc, in_maps, list(range(N_CORES)))
    out = np.concatenate([r["out"] for r in res.results], axis=0)
    return out.astype(np.float32)


# revision 11
# speedup vs baseline: 1.4625x; 1.0119x over previous
"""XNOR-Net++ 3x3 conv (sign(x) (*) sign(w) * alpha*beta*gamma) on 8 TRN2 NeuronCores.

Sharding: data-parallel over batch (32 -> 4 per core), weights/scales replicated.

Per core:
- x and the pre-transposed weight are staged to HBM as bf16 (sign-preserving cast,
  halves DMA); output is written bf16 and upcast on host (conv values are integers
  <= 2304, bf16 rel err < 0.4% << 2e-2 gate)
- weights arrive pre-transposed from host ([i, ky*kx, ob, cb, o] layout),
  binarized on-device to fp8 in one ACT op (no PE transposes)
- sign image: ONE padded fp8 buffer [128, 2, 58, 64] per image (double-buffered,
  borders zeroed once in the preamble); the 9 conv taps read strided windows
  [*, *, t*8+ky : +8, kx : kx+56] directly -- no shifted copies, no per-image
  memsets; sign runs in 7 row-chunks so early row-tiles' matmuls start ASAP
- PE warm-up filler matmuls bridge the DMA-bound startup so the HAM clock gate
  stays at 2.4 GHz when the conv stream begins (cold-start costs ~25us otherwise)
- 3x3 conv = 9 accumulating DoubleRow fp8 matmuls per [128, 448] output tile
  (K=256 via input-channel-block pairing, 2 fp8 weights/PE cell)
- epilogue: single DVE mul with a precomputed alpha*beta*gamma map
  (partition_broadcast + per-partition alpha scale; no fp32 matmuls)
"""

from contextlib import ExitStack

import ml_dtypes
import numpy as np

import concourse.bacc as bacc
import concourse.bass as bass
import concourse.mybir as mybir
import concourse.tile as tile
from concourse.bass_utils import run_bass_kernel_spmd

N_CORES = 8
B, C, H, KS = 32, 256, 56, 3
P = 128
CB = C // P  # input-channel blocks (2)
OB = C // P  # output-channel blocks (2)
HP = H + 2   # padded image rows (58)
WP = 64      # padded image row pitch (cols 0..57 live, 58..63 never read)
R = 8        # output rows per matmul tile
T = H // R   # row tiles per image (7)
NT = R * H   # moving free dim per matmul (448)
HW = H * H   # pixels per image (3136)
RA = 9       # first x chunk (data rows 0..8) -- lands early, unblocks tile t=0
RT = 24      # second x chunk (data rows 9..32)
RB = H - RA - RT  # third x chunk (data rows 33..55)
# sign row-chunks (data-row ranges); chunk 0 reads xA, 1-3 read xT, 4-6 read xB
CHUNKS = [(0, 9), (9, 17), (17, 25), (25, 33), (33, 41), (41, 49), (49, 56)]
N_WARM = 28  # PE warm-up fillers bridging the DMA-bound startup
WSPLIT = 2048  # weight sign chunk boundary (taps 0-3 | 4-8), overlaps w DMA

F32 = mybir.dt.float32
BF16 = mybir.dt.bfloat16
FP8 = mybir.dt.float8e4
DR = mybir.MatmulPerfMode.DoubleRow


def build_conv(tc, out_ap, x_ap, wT_ap, a_ap, b_ap, g_ap, BL):
    nc = tc.nc
    with ExitStack() as ctx:
        const_pool = ctx.enter_context(tc.tile_pool(name="const", bufs=1))
        wpool = ctx.enter_context(tc.tile_pool(name="w", bufs=1))
        xpool = ctx.enter_context(tc.tile_pool(name="x", bufs=2))
        impool = ctx.enter_context(tc.tile_pool(name="img", bufs=1))
        psumpool = ctx.enter_context(tc.tile_pool(name="psum", bufs=4, space="PSUM"))
        opool = ctx.enter_context(tc.tile_pool(name="o", bufs=4))

        # ---- weights first (the startup-critical DMA), split so the ACT sign
        # of taps 0-3 overlaps the DMA of taps 4-8 ----
        w_bf = wpool.tile([P, KS * KS * OB * CB * P], BF16, name="w_bf")
        nc.sync.dma_start(w_bf[:, 0:WSPLIT], wT_ap[:, 0:WSPLIT])
        nc.sync.dma_start(
            w_bf[:, WSPLIT : KS * KS * OB * CB * P],
            wT_ap[:, WSPLIT : KS * KS * OB * CB * P],
        )
        wT2 = wpool.tile([P, KS * KS * OB * CB * P], FP8, name="wT2")
        nc.scalar.sign(wT2[:, 0:WSPLIT], w_bf[:, 0:WSPLIT])
        # second weight-sign half is emitted inside the b==0 loop, after image
        # 0's first row-chunk sign, so tile t=0's taps 0-3 can start early
        # wv[i_low, tap, ob, cb, o]; pair dim cb has byte-step 128 (%16==0)
        wv = wT2.rearrange("p (kk ob cb o) -> p kk ob cb o", kk=KS * KS, ob=OB, cb=CB)

        # ---- PE warm-up fillers: keep HAM at 2.4 GHz until the conv stream ----
        ones_t = const_pool.tile([1, NT], BF16, name="ones_t")
        nc.gpsimd.memset(ones_t, 1.0)
        warm_ps = psumpool.tile([P, NT], F32, name="warm", tag="warm", bufs=1)
        for _ in range(N_WARM):
            nc.tensor.matmul(
                warm_ps, ones_t[0:1, 0:P], ones_t, start=True, stop=True
            )

        # ---- persistent padded sign images; borders zeroed once ----
        ims = [
            impool.tile([P, CB, HP, WP], FP8, name=f"im{i}", tag=f"im{i}")
            for i in range(2)
        ]
        for im in ims:
            nc.gpsimd.memset(im[:, :, 0, 0:58], 0.0)
            nc.gpsimd.memset(im[:, :, HP - 1, 0:58], 0.0)
            nc.gpsimd.memset(im[:, :, 1 : HP - 1, 0], 0.0)
            nc.gpsimd.memset(im[:, :, 1 : HP - 1, 57], 0.0)

        abg = const_pool.tile([P, OB, HW], BF16, name="abg")
        bg_bc = const_pool.tile([P, HW], BF16, name="bg_bc")

        # ---- main loop over local batches ----
        x_v = x_ap.rearrange("b (cb p) h w -> b p cb (h w)", p=P)
        out_v = out_ap.rearrange("b (ob p) h w -> b ob p (h w)", p=P)
        for b in range(BL):
            im = ims[b % 2]
            xA = xpool.tile([P, CB, RA * H], BF16, name="xA", tag="xA")
            nc.sync.dma_start(xA, x_v[b][:, :, 0 : RA * H])
            xT = xpool.tile([P, CB, RT * H], BF16, name="xT", tag="xT")
            nc.sync.dma_start(xT, x_v[b][:, :, RA * H : (RA + RT) * H])
            xB = xpool.tile([P, CB, RB * H], BF16, name="xB", tag="xB")
            nc.sync.dma_start(xB, x_v[b][:, :, (RA + RT) * H : HW])
            views = [
                (0, xA.rearrange("p c (h w) -> p c h w", h=RA)),
                (RA, xT.rearrange("p c (h w) -> p c h w", h=RT)),
                (RA + RT, xB.rearrange("p c (h w) -> p c h w", h=RB)),
            ]
            for ci, (r0, r1) in enumerate(CHUNKS):
                v0, view = next(v for v in reversed(views) if v[0] <= r0)
                nc.scalar.sign(
                    im[:, :, 1 + r0 : 1 + r1, 1 : 1 + H],
                    view[:, :, r0 - v0 : r1 - v0, :],
                )
                if b == 0 and ci == 0:
                    nc.scalar.sign(
                        wT2[:, WSPLIT : KS * KS * OB * CB * P],
                        w_bf[:, WSPLIT : KS * KS * OB * CB * P],
                    )

            if b == 0:
                # tiny scale DMAs + alpha*beta*gamma map. Emitted after the
                # startup-critical dispatches (w, image-0 x) so the Sync engine
                # doesn't delay those; the map is only needed by epilogues, and
                # cps bufs=7 means a late map can't stall the PE.
                a_t = const_pool.tile([P, OB], F32, name="a_t")
                nc.sync.dma_start(
                    a_t, a_ap.rearrange("(ob p) u v -> p (ob u v)", p=P)
                )
                b_t = const_pool.tile([1, H], F32, name="b_t")
                nc.sync.dma_start(b_t, b_ap[0:1, :, 0])
                g_t = const_pool.tile([1, H], F32, name="g_t")
                nc.sync.dma_start(g_t, g_ap[0:1, 0, :])
                bg_row = const_pool.tile([1, HW], BF16, name="bg_row")
                b_rep = b_t[0:1, :].unsqueeze(2).to_broadcast((1, H, H))
                g_rep = g_t[0:1, :].unsqueeze(1).to_broadcast((1, H, H))
                nc.vector.tensor_mul(
                    bg_row.rearrange("a (i j) -> a i j", i=H), b_rep, g_rep
                )
                nc.gpsimd.partition_broadcast(bg_bc, bg_row)
                for ob in range(OB):
                    nc.vector.tensor_scalar_mul(
                        abg[:, ob, :], bg_bc, a_t[:, ob : ob + 1]
                    )

            for ob in range(OB):
                for t in range(T):
                    ps = psumpool.tile([P, NT], F32, name="cps", tag="cps", bufs=7)
                    for kk in range(KS * KS):
                        ky, kx = divmod(kk, KS)
                        rhs = im[:, :, t * R + ky : t * R + ky + R, kx : kx + H]
                        nc.tensor.matmul(
                            ps,
                            wv[:, kk, ob],
                            rhs,
                            start=(kk == 0),
                            stop=(kk == KS * KS - 1),
                            perf_mode=DR,
                        )
                    sl = slice(t * NT, (t + 1) * NT)
                    ot = opool.tile([P, NT], BF16, name="ot")
                    nc.vector.tensor_mul(ot, ps, abg[:, ob, sl])
                    nc.sync.dma_start(out_v[b, ob][:, sl], ot)


def build_nc(BL):
    nc = bacc.Bacc("TRN2", target_bir_lowering=False, debug=False)
    x = nc.dram_tensor("x", [BL, C, H, H], BF16, kind="ExternalInput")
    wT = nc.dram_tensor(
        "weightT", [P, KS * KS * OB * CB * P], BF16, kind="ExternalInput"
    )
    a = nc.dram_tensor("alpha", [C, 1, 1], F32, kind="ExternalInput")
    be = nc.dram_tensor("beta", [1, H, 1], F32, kind="ExternalInput")
    g = nc.dram_tensor("gamma", [1, 1, H], F32, kind="ExternalInput")
    o = nc.dram_tensor("out", [BL, C, H, H], BF16, kind="ExternalOutput")
    with tile.TileContext(nc) as tc:
        build_conv(tc, o.ap(), x.ap(), wT.ap(), a.ap(), be.ap(), g.ap(), BL)
    nc.compile()
    return nc


_nc_cache = {}


def _get_nc(BL):
    if BL not in _nc_cache:
        _nc_cache[BL] = build_nc(BL)
    return _nc_cache[BL]


def _prep(x, weight, alpha, beta, gamma):
    """Build the bass kernel and the per-core input maps."""
    x = np.asarray(x, dtype=np.float32)
    weight = np.asarray(weight, dtype=np.float32)
    alpha = np.ascontiguousarray(np.asarray(alpha, dtype=np.float32))
    beta = np.ascontiguousarray(np.asarray(beta, dtype=np.float32))
    gamma = np.ascontiguousarray(np.asarray(gamma, dtype=np.float32))

    # bf16 staging: sign(bf16(v)) == sign(v) for all practically occurring values
    x_bf = np.ascontiguousarray(x.astype(ml_dtypes.bfloat16))
    # [o, i, ky, kx] -> [i_low, (ky kx), ob, cb, o_low]
    w6 = weight.reshape(OB, P, CB, P, KS, KS)
    wT = np.ascontiguousarray(
        w6.transpose(3, 4, 5, 0, 2, 1).astype(ml_dtypes.bfloat16)
    ).reshape(P, KS * KS * OB * CB * P)

    BL = B // N_CORES
    nc = _get_nc(BL)
    xs = x_bf.reshape(N_CORES, BL, C, H, H)
    in_maps = [
        {"x": xs[c], "weightT": wT, "alpha": alpha, "beta": beta, "gamma": gamma}
        for c in range(N_CORES)
    ]
    return nc, in_maps


def kernel(x, weight, alpha, beta, gamma):
    nc, in_maps = _prep(x, weight, alpha, beta, gamma)
    res = run_bass_kernel_spmd(nc, in_maps, list(range(N_CORES)))
    out = np.concatenate([r["out"] for r in res.results], axis=0)
    return out.astype(np.float32)
